# revision 8
# baseline (speedup 1.0000x reference)
"""Block-circulant linear (MINI_BLOCK=4) via length-4 rFFT factorization on 8 trn2 cores.

Math: out = x @ W^T where W[4y+n, 4x+j] = eigens[y, x, (n-j) mod 4].
In the length-4 DFT domain the circulant contraction factors into 5 real
matmul chains over the block-index axis gx=1024 (Gauss 3-mult for the complex
bin; ~13x fewer FLOPs than the dense 4096^3 matmul):
  X0 = x0+x1+x2+x3, X1 = (x0-x2) + i(x3-x1), X2 = x0-x1+x2-x3  (per block of 4)
  g1 = (X1r+X1i)E1r, g2 = X1r(E1i-E1r), g3 = X1i(E1r+E1i)
  Y0 = X0 E0, Y2 = X2 E2, Y1r = g1-g3, Y1i = g1+g2
  o0 = Y0+Y1r+Y2, o1 = Y0-Y1i-Y2, o2 = Y0-Y1r+Y2, o3 = Y0+Y1i-Y2  (scales in E)

Sharding: data-parallel over batch, 512 rows per core; E replicated.

Device mapping (v2): E-stationary / X-moving. The forward DFT of x (cheap
butterflies) and of eigens is precomputed on host and shipped as bf16; on
device, for each 128-wide y-chunk c the five bins accumulate
  ps[k] = sum_xc es[k][c][:,xc,:].T @ xm[k][:,xc,:]   -> [128 y, 512 b] PSUM
(8 matmuls of 128x128x512 bf16 per bin), then ACT/DVE drain the five banks
into the inverse-DFT combines (bf16) and one [128, 4, 512] bf16 tile is
stored per chunk; the host de-interleaves [c,y,n,b] -> [b, 4*(128c+y)+n].
All DMA uses >=2KB contiguous lines per partition (descriptor-rate limit).
PSUM: one full bank per bin accumulator, bufs g1/y0:1, g2/y2/g3:2 = 8 banks;
g1/y0 are freed first by the ACT copies so single-buffering them is safe.
"""
import numpy as np

B, IN, OUT, BLK = 4096, 4096, 4096, 4
GX, GY = IN // BLK, OUT // BLK        # 1024, 1024
NCORES = 8
BS = B // NCORES                      # 512 batch rows per core
XC = GX // 128                        # 8 x-chunks (contraction)
YC = GY // 128                        # 8 y-chunks (output)
BINS = ("g1", "y0", "g2", "y2", "g3")
PSUM_BUFS = {"g1": 1, "y0": 1, "g2": 2, "y2": 2, "g3": 2}

_cache = {}


def _build_nc():
    from concourse import bacc
    import concourse.mybir as mybir
    from concourse.tile import TileContext

    f32 = mybir.dt.float32
    bf16 = mybir.dt.bfloat16

    nc = bacc.Bacc("TRN2", target_bir_lowering=False, debug=False,
                   enable_asserts=False, num_devices=NCORES)
    # X bins fused, host-DFT'd, in xc-pair quarters: [q, p, bin, xc2, b].
    xq_d = nc.dram_tensor("xq", [4, 128, 5, 2, BS], bf16, kind="ExternalInput")
    # E chunk 0, all five bins fused into one early call: [p, bin, xc, y].
    ec0_d = nc.dram_tensor("ec0", [128, 5, XC, 128], bf16, kind="ExternalInput")
    # E bins for chunks 1..7, [y-chunk, p, xc, y]: 2KB lines per partition.
    eh_d = [nc.dram_tensor(f"e{k}", [YC - 1, 128, XC, 128], bf16,
                           kind="ExternalInput")
            for k in range(5)]
    # out [y-chunk, y, n, b] bf16: 4KB lines; host de-interleaves.
    od_d = nc.dram_tensor("out", [YC, 128, 4, BS], bf16, kind="ExternalOutput")

    with TileContext(nc) as tc:
        with (
            tc.tile_pool(name="xm", bufs=1) as xp,
            tc.tile_pool(name="es", bufs=1) as ep,
            tc.tile_pool(name="tv", bufs=2) as tvp,
            tc.tile_pool(name="abcd", bufs=2) as ab,
            tc.tile_pool(name="otp", bufs=3) as op_,
            tc.tile_pool(name="ps", bufs=1, space="PSUM") as mps,
        ):
            # Persistent SBUF residency: X 40KB/part, E 80KB/part.
            xm = xp.tile([128, 5, XC, BS], bf16, tag="xm", name="xm")
            e0 = ep.tile([128, 5, XC, 128], bf16, tag="ec0", name="e0")
            es = [[ep.tile([128, XC, 128], bf16, tag=f"e{k}_{c}", name=f"es{k}_{c}")
                   for c in range(1, YC)] for k in range(5)]

            # DMA issue order = arrival priority. Chunk-0 E (one fused call)
            # and the four X quarters on the SP ring; later E chunks on the
            # gpsimd SWDGE ring.
            nc.sync.dma_start(out=e0, in_=ec0_d[:, :])
            for q in range(4):
                nc.sync.dma_start(out=xm[:, :, 2 * q:2 * q + 2, :], in_=xq_d[q])
            for c in range(1, YC):
                for k in range(5):
                    nc.gpsimd.dma_start(out=es[k][c - 1], in_=eh_d[k][c - 1])

            def make_ps(c):
                return {k: mps.tile([128, BS], f32, tag=f"ps_{k}",
                                    name=f"ps_{k}", bufs=PSUM_BUFS[k])
                        for k in BINS}

            def mm(ps, c, xc, k, st, sp):
                ki = BINS.index(k)
                stat = e0[:, ki, xc, :] if c == 0 else es[ki][c - 1][:, xc, :]
                nc.tensor.matmul(ps[k], stat, xm[:, ki, xc, :],
                                 start=st, stop=sp, skip_group_check=True)

            def drains(ps, c):
                # Inverse DFT: ACT drains g1/y0 (frees their single banks
                # fast); DVE reads one PSUM operand per op, combines in bf16.
                v_ = tvp.tile([128, BS], f32, tag="v", name="v_")
                t_ = tvp.tile([128, BS], f32, tag="t", name="t_")
                a_ = ab.tile([128, BS], bf16, tag="a", name="a_")
                b_ = ab.tile([128, BS], bf16, tag="b", name="b_")
                c_ = ab.tile([128, BS], bf16, tag="c", name="c_")
                d_ = ab.tile([128, BS], bf16, tag="d", name="d_")
                ot = op_.tile([128, 4, BS], bf16, tag="ot", name="ot")
                nc.scalar.copy(out=v_, in_=ps["g1"])              # frees g1
                nc.scalar.copy(out=t_, in_=ps["y0"])              # frees y0
                nc.vector.tensor_add(out=d_, in0=v_, in1=ps["g2"])  # Y1i
                nc.vector.tensor_sub(out=c_, in0=v_, in1=ps["g3"])  # Y1r
                nc.vector.tensor_add(out=a_, in0=t_, in1=ps["y2"])  # Y0+Y2
                nc.vector.tensor_sub(out=b_, in0=t_, in1=ps["y2"])  # Y0-Y2
                nc.vector.tensor_add(out=ot[:, 0, :], in0=a_, in1=c_)
                nc.vector.tensor_sub(out=ot[:, 2, :], in0=a_, in1=c_)
                nc.vector.tensor_sub(out=ot[:, 1, :], in0=b_, in1=d_)
                nc.vector.tensor_add(out=ot[:, 3, :], in0=b_, in1=d_)
                nc.sync.dma_start(out=od_d[c], in_=ot)

            # Head: interleave chunk-1's double-buffered bins (xc 0-3, h0
            # data) into chunk-0's stream so the PE has work while the
            # second half of X is still arriving.
            ps0, ps1 = make_ps(0), make_ps(1)
            for xc in range(4):
                for k in BINS:
                    mm(ps0, 0, xc, k, xc == 0, False)
            for xc in range(4):
                for k in ("g2", "y2", "g3"):
                    mm(ps1, 1, xc, k, xc == 0, False)
            for xc in range(4, XC):
                for k in BINS:
                    mm(ps0, 0, xc, k, False, xc == XC - 1)
            drains(ps0, 0)
            for xc in range(XC):
                for k in ("g1", "y0"):
                    mm(ps1, 1, xc, k, xc == 0, xc == XC - 1)
            for xc in range(4, XC):
                for k in ("g2", "y2", "g3"):
                    mm(ps1, 1, xc, k, False, xc == XC - 1)
            drains(ps1, 1)

            for c in range(2, YC - 1):
                ps = make_ps(c)
                for xc in range(XC):
                    st, sp = xc == 0, xc == XC - 1
                    for k in BINS:
                        mm(ps, c, xc, k, st, sp)
                drains(ps, c)

            # Tail chunk: stagger bin completion (y0/y2/g1 rotate first, then
            # g2/g3) so drains and per-phase stores overlap this chunk's own
            # matmuls; only c = v-g3 and o0/o2 trail the last matmul.
            c = YC - 1
            ps = make_ps(c)
            for xc in range(XC):
                for k in ("y0", "y2", "g1"):
                    mm(ps, c, xc, k, xc == 0, xc == XC - 1)
            for xc in range(XC):
                for k in ("g2", "g3"):
                    mm(ps, c, xc, k, xc == 0, xc == XC - 1)
            v_ = tvp.tile([128, BS], f32, tag="v", name="v_")
            t_ = tvp.tile([128, BS], f32, tag="t", name="t_")
            a_ = ab.tile([128, BS], bf16, tag="a", name="a_")
            b_ = ab.tile([128, BS], bf16, tag="b", name="b_")
            c_ = ab.tile([128, BS], bf16, tag="c", name="c_")
            d_ = ab.tile([128, BS], bf16, tag="d", name="d_")
            ot = op_.tile([128, 4, BS], bf16, tag="ot", name="ot")
            nc.scalar.copy(out=t_, in_=ps["y0"])
            nc.scalar.copy(out=v_, in_=ps["g1"])
            nc.vector.tensor_add(out=a_, in0=t_, in1=ps["y2"])
            nc.vector.tensor_sub(out=b_, in0=t_, in1=ps["y2"])
            nc.vector.tensor_add(out=d_, in0=v_, in1=ps["g2"])
            nc.vector.tensor_sub(out=ot[:, 1, :], in0=b_, in1=d_)
            nc.vector.tensor_add(out=ot[:, 3, :], in0=b_, in1=d_)
            nc.sync.dma_start(out=od_d[c][:, 1, :], in_=ot[:, 1, :])
            nc.sync.dma_start(out=od_d[c][:, 3, :], in_=ot[:, 3, :])
            nc.vector.tensor_sub(out=c_, in0=v_, in1=ps["g3"])
            nc.vector.tensor_add(out=ot[:, 0, :], in0=a_, in1=c_)
            nc.vector.tensor_sub(out=ot[:, 2, :], in0=a_, in1=c_)
            nc.sync.dma_start(out=od_d[c][:, 0, :], in_=ot[:, 0, :])
            nc.sync.dma_start(out=od_d[c][:, 2, :], in_=ot[:, 2, :])
    nc.compile()
    return nc


def _prep_eigens(eigens):
    """eigens (gy, gx, 4) -> fused chunk-0 tensor [128, 5, XC, 128] ("ec0")
    plus five [YC-1, 128, XC, 128] bf16 E-bins for chunks 1..7,
    [x, y]-oriented with irfft scale factors folded in."""
    import ml_dtypes
    e = np.ascontiguousarray(np.asarray(eigens).transpose(1, 0, 2)).astype(np.float32)
    e0 = ((e[..., 0] + e[..., 2]) + (e[..., 1] + e[..., 3])) * 0.25
    e2 = ((e[..., 0] + e[..., 2]) - (e[..., 1] + e[..., 3])) * 0.25
    e1r = (e[..., 0] - e[..., 2]) * 0.5
    e1i = (e[..., 3] - e[..., 1]) * 0.5
    mats = {"g1": e1r, "y0": e0, "g2": e1i - e1r, "y2": e2, "g3": e1r + e1i}

    def chunk(m):  # [1024x, 1024y] -> [YC, 128p, XC, 128y]
        return m.reshape(XC, 128, YC, 128).transpose(2, 1, 0, 3)
    ch = [chunk(mats[k]) for k in BINS]
    ec0 = np.ascontiguousarray(
        np.stack([m[0] for m in ch], axis=1)).astype(ml_dtypes.bfloat16)
    out = {"ec0": ec0}
    for ki in range(5):
        out[f"e{ki}"] = np.ascontiguousarray(ch[ki][1:]).astype(ml_dtypes.bfloat16)
    return out


def _prep_x(xs):
    """x shard [BS, 4096] f32 -> fused forward-DFT bins in xc-pair quarters:
    [4, 128, 5, 2, BS] bf16 ("xq")."""
    import ml_dtypes
    xb = xs.reshape(BS, GX, 4)
    x0, x1, x2, x3 = (xb[..., j] for j in range(4))
    x1r = x0 - x2
    x1i = x3 - x1
    mats = {"g1": x1r + x1i, "y0": x0 + x1 + x2 + x3, "g2": x1r,
            "y2": x0 - x1 + x2 - x3, "g3": x1i}
    # [BS, 1024x] -> [4q, 2xc2, 128p, BS] per bin, stacked on axis 2.
    ch = [mats[k].T.reshape(4, 2, 128, BS) for k in BINS]
    xq = np.stack(ch, axis=2).transpose(0, 3, 2, 1, 4)  # [q, p, bin, xc2, b]
    return {"xq": np.ascontiguousarray(xq).astype(ml_dtypes.bfloat16)}


def _in_maps(x, eigens):
    x = np.ascontiguousarray(np.asarray(x), dtype=np.float32)
    emaps = _prep_eigens(eigens)
    return [dict(_prep_x(x[c * BS:(c + 1) * BS]), **emaps) for c in range(NCORES)]


def _assemble(results):
    # od [YC, 128y, 4n, BS b] bf16 -> [BS, 4096] f32 per core.
    return np.concatenate(
        [np.asarray(r["out"]).transpose(3, 0, 1, 2).reshape(BS, OUT).astype(np.float32)
         for r in results], axis=0)


def kernel(x, eigens):
    from concourse.bass_utils import run_bass_kernel_spmd

    if "nc" not in _cache:
        _cache["nc"] = _build_nc()
    res = run_bass_kernel_spmd(_cache["nc"], _in_maps(x, eigens),
                               core_ids=list(range(NCORES)))
    return _assemble(res.results)


# revision 11
# speedup vs baseline: 1.1454x; 1.1454x over previous
"""Block-circulant linear (MINI_BLOCK=4) via length-4 rFFT factorization on 8 trn2 cores.

Math: out = x @ W^T where W[4y+n, 4x+j] = eigens[y, x, (n-j) mod 4].
In the length-4 DFT domain the circulant contraction factors into 5 real
matmul chains over the block-index axis gx=1024 (Gauss 3-mult for the complex
bin; ~13x fewer FLOPs than the dense 4096^3 matmul):
  X0 = x0+x1+x2+x3, X1 = (x0-x2) + i(x3-x1), X2 = x0-x1+x2-x3  (per block of 4)
  g1 = (X1r+X1i)E1r, g2 = X1r(E1i-E1r), g3 = X1i(E1r+E1i)
  Y0 = X0 E0, Y2 = X2 E2, Y1r = g1-g3, Y1i = g1+g2
  o0 = Y0+Y1r+Y2, o1 = Y0-Y1i-Y2, o2 = Y0-Y1r+Y2, o3 = Y0+Y1i-Y2  (scales in E)

Sharding: data-parallel over batch, 512 rows per core; E replicated.

Device mapping (v2): E-stationary / X-moving. The forward DFT of x (cheap
butterflies) and of eigens is precomputed on host and shipped as bf16; on
device, for each 128-wide y-chunk c the five bins accumulate
  ps[k] = sum_xc es[k][c][:,xc,:].T @ xm[k][:,xc,:]   -> [128 y, 512 b] PSUM
(8 matmuls of 128x128x512 bf16 per bin), then ACT/DVE drain the five banks
into the inverse-DFT combines (bf16) and one [128, 4, 512] bf16 tile is
stored per chunk; the host de-interleaves [c,y,n,b] -> [b, 4*(128c+y)+n].
All DMA uses >=2KB contiguous lines per partition (descriptor-rate limit).
PSUM: one full bank per bin accumulator, bufs g1/y0:1, g2/y2/g3:2 = 8 banks;
g1/y0 are freed first by the ACT copies so single-buffering them is safe.
"""
import numpy as np

B, IN, OUT, BLK = 4096, 4096, 4096, 4
GX, GY = IN // BLK, OUT // BLK        # 1024, 1024
NCORES = 8
BS = B // NCORES                      # 512 batch rows per core
XC = GX // 128                        # 8 x-chunks (contraction)
YC = GY // 128                        # 8 y-chunks (output)
BINS = ("g1", "y0", "g2", "y2", "g3")
PSUM_BUFS = {"g1": 1, "y0": 1, "g2": 2, "y2": 2, "g3": 2}

_cache = {}


def _build_nc():
    from concourse import bacc
    import concourse.mybir as mybir
    from concourse.tile import TileContext

    f32 = mybir.dt.float32
    bf16 = mybir.dt.bfloat16

    nc = bacc.Bacc("TRN2", target_bir_lowering=False, debug=False,
                   enable_asserts=False, num_devices=NCORES)
    # X bins, host-DFT'd, per (bin, xc-half): 4KB lines, 128 desc per call.
    xh_d = [nc.dram_tensor(f"x{k}", [2, 128, 4, BS], bf16, kind="ExternalInput")
            for k in range(5)]
    # E chunk 0, all five bins fused into one early call: [p, bin, xc, y].
    ec0_d = nc.dram_tensor("ec0", [128, 5, XC, 128], bf16, kind="ExternalInput")
    # E bins for chunks 1..7, [y-chunk, p, xc, y]: 2KB lines per partition.
    eh_d = [nc.dram_tensor(f"e{k}", [YC - 1, 128, XC, 128], bf16,
                           kind="ExternalInput")
            for k in range(5)]
    # out [y-chunk, y, n, b] bf16: 4KB lines; host de-interleaves.
    od_d = nc.dram_tensor("out", [YC, 128, 4, BS], bf16, kind="ExternalOutput")

    with TileContext(nc) as tc:
        with (
            tc.tile_pool(name="xm", bufs=1) as xp,
            tc.tile_pool(name="es", bufs=1) as ep,
            tc.tile_pool(name="tv", bufs=2) as tvp,
            tc.tile_pool(name="abcd", bufs=2) as ab,
            tc.tile_pool(name="otp", bufs=3) as op_,
            tc.tile_pool(name="ps", bufs=1, space="PSUM") as mps,
        ):
            # Persistent SBUF residency: X 40KB/part, E 80KB/part.
            xm = xp.tile([128, 5, XC, BS], bf16, tag="xm", name="xm")
            e0 = ep.tile([128, 5, XC, 128], bf16, tag="ec0", name="e0")
            es = [[ep.tile([128, XC, 128], bf16, tag=f"e{k}_{c}", name=f"es{k}_{c}")
                   for c in range(1, YC)] for k in range(5)]

            # DMA issue order = arrival priority. Chunk-0 E (one fused call)
            # and the four X quarters on the SP ring; later E chunks on the
            # gpsimd SWDGE ring.
            nc.sync.dma_start(out=e0, in_=ec0_d[:, :])
            for h in range(2):
                for k in range(5):
                    nc.sync.dma_start(out=xm[:, k, 4 * h:4 * h + 4, :],
                                      in_=xh_d[k][h])
            for c in range(1, YC):
                for k in range(5):
                    nc.gpsimd.dma_start(out=es[k][c - 1], in_=eh_d[k][c - 1])

            def make_ps(c):
                return {k: mps.tile([128, BS], f32, tag=f"ps_{k}",
                                    name=f"ps_{k}", bufs=PSUM_BUFS[k])
                        for k in BINS}

            def mm(ps, c, xc, k, st, sp):
                ki = BINS.index(k)
                stat = e0[:, ki, xc, :] if c == 0 else es[ki][c - 1][:, xc, :]
                nc.tensor.matmul(ps[k], stat, xm[:, ki, xc, :],
                                 start=st, stop=sp, skip_group_check=True)

            def drains(ps, c):
                # Inverse DFT: ACT drains g1/y0 (frees their single banks
                # fast); DVE reads one PSUM operand per op, combines in bf16.
                v_ = tvp.tile([128, BS], f32, tag="v", name="v_")
                t_ = tvp.tile([128, BS], f32, tag="t", name="t_")
                a_ = ab.tile([128, BS], bf16, tag="a", name="a_")
                b_ = ab.tile([128, BS], bf16, tag="b", name="b_")
                c_ = ab.tile([128, BS], bf16, tag="c", name="c_")
                d_ = ab.tile([128, BS], bf16, tag="d", name="d_")
                ot = op_.tile([128, 4, BS], bf16, tag="ot", name="ot")
                nc.scalar.copy(out=v_, in_=ps["g1"])              # frees g1
                nc.scalar.copy(out=t_, in_=ps["y0"])              # frees y0
                nc.vector.tensor_add(out=d_, in0=v_, in1=ps["g2"])  # Y1i
                nc.vector.tensor_sub(out=c_, in0=v_, in1=ps["g3"])  # Y1r
                nc.vector.tensor_add(out=a_, in0=t_, in1=ps["y2"])  # Y0+Y2
                nc.vector.tensor_sub(out=b_, in0=t_, in1=ps["y2"])  # Y0-Y2
                nc.vector.tensor_add(out=ot[:, 0, :], in0=a_, in1=c_)
                nc.vector.tensor_sub(out=ot[:, 2, :], in0=a_, in1=c_)
                nc.vector.tensor_sub(out=ot[:, 1, :], in0=b_, in1=d_)
                nc.vector.tensor_add(out=ot[:, 3, :], in0=b_, in1=d_)
                nc.sync.dma_start(out=od_d[c], in_=ot)

            # Head: interleave chunk-1's double-buffered bins (xc 0-3, h0
            # data) into chunk-0's stream so the PE has work while the
            # second half of X is still arriving.
            ps0, ps1 = make_ps(0), make_ps(1)
            for xc in range(4):
                for k in BINS:
                    mm(ps0, 0, xc, k, xc == 0, False)
            for xc in range(4):
                for k in ("g2", "y2", "g3"):
                    mm(ps1, 1, xc, k, xc == 0, False)
            for xc in range(4, XC):
                for k in BINS:
                    mm(ps0, 0, xc, k, False, xc == XC - 1)
            drains(ps0, 0)
            for xc in range(XC):
                for k in ("g1", "y0"):
                    mm(ps1, 1, xc, k, xc == 0, xc == XC - 1)
            for xc in range(4, XC):
                for k in ("g2", "y2", "g3"):
                    mm(ps1, 1, xc, k, False, xc == XC - 1)
            drains(ps1, 1)

            for c in range(2, YC - 1):
                ps = make_ps(c)
                for xc in range(XC):
                    st, sp = xc == 0, xc == XC - 1
                    for k in BINS:
                        mm(ps, c, xc, k, st, sp)
                drains(ps, c)

            # Tail chunk: stagger bin completion (y0/y2/g1 rotate first, then
            # g2/g3) so drains and per-phase stores overlap this chunk's own
            # matmuls; only c = v-g3 and o0/o2 trail the last matmul.
            c = YC - 1
            ps = make_ps(c)
            for xc in range(XC):
                for k in ("y0", "y2", "g1"):
                    mm(ps, c, xc, k, xc == 0, xc == XC - 1)
            for xc in range(XC):
                for k in ("g2", "g3"):
                    mm(ps, c, xc, k, xc == 0, xc == XC - 1)
            v_ = tvp.tile([128, BS], f32, tag="v", name="v_")
            t_ = tvp.tile([128, BS], f32, tag="t", name="t_")
            a_ = ab.tile([128, BS], bf16, tag="a", name="a_")
            b_ = ab.tile([128, BS], bf16, tag="b", name="b_")
            c_ = ab.tile([128, BS], bf16, tag="c", name="c_")
            d_ = ab.tile([128, BS], bf16, tag="d", name="d_")
            ot = op_.tile([128, 4, BS], bf16, tag="ot", name="ot")
            nc.scalar.copy(out=t_, in_=ps["y0"])
            nc.scalar.copy(out=v_, in_=ps["g1"])
            nc.vector.tensor_add(out=a_, in0=t_, in1=ps["y2"])
            nc.vector.tensor_sub(out=b_, in0=t_, in1=ps["y2"])
            nc.vector.tensor_add(out=d_, in0=v_, in1=ps["g2"])
            nc.vector.tensor_sub(out=ot[:, 1, :], in0=b_, in1=d_)
            nc.vector.tensor_add(out=ot[:, 3, :], in0=b_, in1=d_)
            nc.sync.dma_start(out=od_d[c][:, 1, :], in_=ot[:, 1, :])
            nc.sync.dma_start(out=od_d[c][:, 3, :], in_=ot[:, 3, :])
            nc.vector.tensor_sub(out=c_, in0=v_, in1=ps["g3"])
            nc.vector.tensor_add(out=ot[:, 0, :], in0=a_, in1=c_)
            nc.vector.tensor_sub(out=ot[:, 2, :], in0=a_, in1=c_)
            nc.sync.dma_start(out=od_d[c][:, 0, :], in_=ot[:, 0, :])
            nc.sync.dma_start(out=od_d[c][:, 2, :], in_=ot[:, 2, :])
    nc.compile()
    return nc


def _prep_eigens(eigens):
    """eigens (gy, gx, 4) -> fused chunk-0 tensor [128, 5, XC, 128] ("ec0")
    plus five [YC-1, 128, XC, 128] bf16 E-bins for chunks 1..7,
    [x, y]-oriented with irfft scale factors folded in."""
    import ml_dtypes
    e = np.ascontiguousarray(np.asarray(eigens).transpose(1, 0, 2)).astype(np.float32)
    e0 = ((e[..., 0] + e[..., 2]) + (e[..., 1] + e[..., 3])) * 0.25
    e2 = ((e[..., 0] + e[..., 2]) - (e[..., 1] + e[..., 3])) * 0.25
    e1r = (e[..., 0] - e[..., 2]) * 0.5
    e1i = (e[..., 3] - e[..., 1]) * 0.5
    mats = {"g1": e1r, "y0": e0, "g2": e1i - e1r, "y2": e2, "g3": e1r + e1i}

    def chunk(m):  # [1024x, 1024y] -> [YC, 128p, XC, 128y]
        return m.reshape(XC, 128, YC, 128).transpose(2, 1, 0, 3)
    ch = [chunk(mats[k]) for k in BINS]
    ec0 = np.ascontiguousarray(
        np.stack([m[0] for m in ch], axis=1)).astype(ml_dtypes.bfloat16)
    out = {"ec0": ec0}
    for ki in range(5):
        out[f"e{ki}"] = np.ascontiguousarray(ch[ki][1:]).astype(ml_dtypes.bfloat16)
    return out


def _prep_x(xs):
    """x shard [BS, 4096] f32 -> fused forward-DFT bins in xc-pair quarters:
    [4, 128, 5, 2, BS] bf16 ("xq")."""
    import ml_dtypes
    xb = xs.reshape(BS, GX, 4)
    x0, x1, x2, x3 = (xb[..., j] for j in range(4))
    x1r = x0 - x2
    x1i = x3 - x1
    mats = {"g1": x1r + x1i, "y0": x0 + x1 + x2 + x3, "g2": x1r,
            "y2": x0 - x1 + x2 - x3, "g3": x1i}

    def chunk(m):  # [BS, 1024x] -> [2h, 128p, 4xc, BS]
        return np.ascontiguousarray(
            m.T.reshape(2, 4, 128, BS).transpose(0, 2, 1, 3)).astype(ml_dtypes.bfloat16)
    return {f"x{ki}": chunk(mats[k]) for ki, k in enumerate(BINS)}


def _in_maps(x, eigens):
    x = np.ascontiguousarray(np.asarray(x), dtype=np.float32)
    emaps = _prep_eigens(eigens)
    return [dict(_prep_x(x[c * BS:(c + 1) * BS]), **emaps) for c in range(NCORES)]


def _assemble(results):
    # od [YC, 128y, 4n, BS b] bf16 -> [BS, 4096] f32 per core.
    return np.concatenate(
        [np.asarray(r["out"]).transpose(3, 0, 1, 2).reshape(BS, OUT).astype(np.float32)
         for r in results], axis=0)


def kernel(x, eigens):
    from concourse.bass_utils import run_bass_kernel_spmd

    if "nc" not in _cache:
        _cache["nc"] = _build_nc()
    res = run_bass_kernel_spmd(_cache["nc"], _in_maps(x, eigens),
                               core_ids=list(range(NCORES)))
    return _assemble(res.results)
